# revision 1
# baseline (speedup 1.0000x reference)
"""Trainium2 Bass kernel for nn_MHAAttention (LayerNorm2d + MHA w/ rel-pos bias + residual).

Sharding: data-parallel over batch — 8 batch elements, one per NeuronCore.
No collectives needed.

Per-core device pipeline (all fp32):
  x (C=512 part-tiles, T=1024 free)  [channels on partitions]
  LN stats via ones-matmul (replicated across partitions), apply on DVE
  Q,K in (d part, t free);  V in (t part, d free) with per-head [v|1] augment
  scores computed TRANSPOSED per head:  sT[j,i] = sum_d k[j,d] q[i,d]  (K=64 matmul)
  rel-pos bias added from a host-precomputed sliding "strip" (block-Toeplitz
  structure of rel[REL_IDX] means each (head, key-tile) bias block is a
  contiguous slice of a (128, 1920) strip)
  exp on ScalarE (values bounded, no max-subtraction needed)
  attn@V: oT[d,i] = sum_j v_aug[j,d] aT[j,i] accumulated over j-tiles; the
  augmented ones-column yields the softmax denominator Z in row 64
  normalize, project back (K=64 per-head chunks), add bias + residual, DMA out.
"""

import sys

for _p in ("/opt/trn_rl_repo",):
    if _p not in sys.path:
        sys.path.insert(0, _p)

from contextlib import ExitStack

import numpy as np

import concourse.bass as bass
import concourse.mybir as mybir
import concourse.tile as tile
from concourse.bass_utils import run_bass_kernel_spmd

F32 = mybir.dt.float32
AF = mybir.ActivationFunctionType
OP = mybir.AluOpType

B = 8
CH = 512
H = W = 32
NT = H * W          # 1024 tokens
HEADS = 8
HD = 64
EPS = 1e-6
P = 128
CT = CH // P        # 4 channel tiles
TT = NT // P        # 8 token tiles
IC = NT // 512      # 2 free-dim chunks of 512
STRIP_W = 60 * 32   # 1920


def _build_strips(rel: np.ndarray) -> np.ndarray:
    """(3969, 8) rel table -> (8, 128, 1920) bias strips.

    strip[h, 32*jh_l + jw, 32*g + iw] = T_h[g - jh_l + 3, iw - jw + 31]
    where T_h = rel[:, h].reshape(63, 63).
    bias.T block for key-tile jt is then strip[:, (28-4*jt)*32 : +1024].
    """
    T = rel.reshape(63, 63, HEADS)  # [a, b, h]
    jh_l = np.arange(4)[:, None, None, None]
    jw = np.arange(32)[None, :, None, None]
    g = np.arange(60)[None, None, :, None]
    iw = np.arange(32)[None, None, None, :]
    a = g - jh_l + 3          # in [0,62]
    b = iw - jw + 31          # in [0,62]
    a_b, b_b = np.broadcast_arrays(a, b)
    out = T[a_b, b_b, :]      # (4, 32, 60, 32, 8)
    out = np.ascontiguousarray(np.moveaxis(out, -1, 0)).reshape(HEADS, 128, STRIP_W)
    return out.astype(np.float32)


def _build_nc() -> bass.Bass:
    nc = bass.Bass()

    x_d = nc.declare_dram_parameter("x", [CH, NT], F32, isOutput=False)
    wqT_d = nc.declare_dram_parameter("wqT", [CH, CH], F32, isOutput=False)
    wkT_d = nc.declare_dram_parameter("wkT", [CH, CH], F32, isOutput=False)
    wvT_d = nc.declare_dram_parameter("wvT", [CH, CH], F32, isOutput=False)
    wpP_d = nc.declare_dram_parameter("wpP", [HD, HEADS, CH], F32, isOutput=False)
    bqk_d = nc.declare_dram_parameter("bqk", [2, CH], F32, isOutput=False)
    brow_d = nc.declare_dram_parameter("brow", [2, CH], F32, isOutput=False)
    strips_d = nc.declare_dram_parameter("strips", [HEADS, P, STRIP_W], F32, isOutput=False)
    y_d = nc.declare_dram_parameter("y", [CH, NT], F32, isOutput=True)

    with tile.TileContext(nc) as tc, ExitStack() as ctx:
        singles = ctx.enter_context(tc.tile_pool(name="singles", bufs=1))
        work = ctx.enter_context(tc.tile_pool(name="work", bufs=2))
        strip_pool = ctx.enter_context(tc.tile_pool(name="strip_pool", bufs=2))
        at_pool = ctx.enter_context(tc.tile_pool(name="at_pool", bufs=3))
        # PSUM budget (8 banks): psA big (128,1024)x2bufs = 4 banks shared by
        # LN-stats and scores; psB (128,512)x2 = 2 banks for qkv/proj; ps_o 2.
        psA = ctx.enter_context(tc.tile_pool(name="psA", bufs=2, space="PSUM"))
        psB = ctx.enter_context(tc.tile_pool(name="psB", bufs=2, space="PSUM"))
        ps_o = ctx.enter_context(tc.tile_pool(name="ps_o", bufs=1, space="PSUM"))

        # ---------- persistent SBUF ----------
        xn_sb = singles.tile([P, CT, NT], F32)       # x, overwritten by LN output
        qT_sb = singles.tile([P, CT, NT], F32)       # (d part, t free)
        kT_sb = singles.tile([P, CT, NT], F32)
        v_sb = singles.tile([P, TT, HEADS * (HD + 1)], F32)  # per head [v(64) | 1]
        oT_sb = singles.tile([HD, HEADS, NT], F32)   # per-head oT at partitions 0..63
        wpP_sb = singles.tile([HD, HEADS, CH], F32)
        bqk_sb = singles.tile([P, 2, CT], F32)       # per-partition bias cols for q,k
        brow_sb = singles.tile([1, 2, CH], F32)      # bv_eff, bp rows
        ones_mat = singles.tile([P, P], F32)
        ones_row = singles.tile([1, NT], F32)

        nc.vector.memset(ones_mat[:], 1.0)
        nc.vector.memset(ones_row[:], 1.0)
        nc.sync.dma_start(wpP_sb[:], wpP_d[:])
        nc.sync.dma_start(bqk_sb[:], bqk_d.rearrange("i (o p) -> p i o", p=P))
        nc.sync.dma_start(brow_sb[:], brow_d[None, :, :])

        # ones columns of v_aug
        v_view = v_sb[:].rearrange("p tt (h w) -> p tt h w", w=HD + 1)
        nc.vector.memset(v_view[:, :, :, HD : HD + 1], 1.0)

        nc.sync.dma_start(xn_sb[:], x_d.rearrange("(ct p) t -> p ct t", p=P))

        # ---------- phase 1: LayerNorm (stats replicated via ones-matmul) ----------
        with tc.tile_pool(name="ln_pool", bufs=1) as lnp:
            sum_ps = psA.tile([P, NT], F32, tag="big")
            sq_ps = psA.tile([P, NT], F32, tag="big")
            for ct in range(CT):
                x2 = lnp.tile([P, NT], F32, name=f"x2_{ct}", tag="x2")
                nc.scalar.activation(out=x2[:], in_=xn_sb[:, ct], func=AF.Square)
                for ic in range(IC):
                    sl = slice(ic * 512, ic * 512 + 512)
                    nc.tensor.matmul(sum_ps[:, sl], lhsT=ones_mat[:], rhs=xn_sb[:, ct, sl],
                                     start=(ct == 0), stop=(ct == CT - 1))
                    nc.tensor.matmul(sq_ps[:, sl], lhsT=ones_mat[:], rhs=x2[:, sl],
                                     start=(ct == 0), stop=(ct == CT - 1))

            mu = lnp.tile([P, NT], F32)
            rs = lnp.tile([P, NT], F32)
            ve = lnp.tile([P, NT], F32)
            nwt = lnp.tile([P, NT], F32)
            nc.scalar.activation(out=mu[:], in_=sum_ps[:], func=AF.Copy, scale=1.0 / CH)
            nc.scalar.activation(out=ve[:], in_=sq_ps[:], func=AF.Copy, scale=1.0 / CH)
            nc.vector.tensor_tensor(out=nwt[:], in0=mu[:], in1=mu[:], op=OP.mult)
            nc.vector.tensor_tensor(out=ve[:], in0=ve[:], in1=nwt[:], op=OP.subtract)
            nc.vector.tensor_scalar_add(out=ve[:], in0=ve[:], scalar1=float(EPS))
            nc.scalar.activation(out=rs[:], in_=ve[:], func=AF.Sqrt)
            nc.vector.reciprocal(out=rs[:], in_=rs[:])
            # one Newton step: rs *= 1.5 - 0.5 * ve * rs^2  (guards vs ACT table error)
            nc.vector.tensor_tensor(out=nwt[:], in0=rs[:], in1=rs[:], op=OP.mult)
            nc.vector.tensor_tensor(out=nwt[:], in0=nwt[:], in1=ve[:], op=OP.mult)
            nc.vector.tensor_scalar(out=nwt[:], in0=nwt[:], scalar1=-0.5, scalar2=1.5,
                                    op0=OP.mult, op1=OP.add)
            nc.vector.tensor_tensor(out=rs[:], in0=rs[:], in1=nwt[:], op=OP.mult)

            for ct in range(CT):
                nc.vector.tensor_tensor(out=xn_sb[:, ct], in0=xn_sb[:, ct], in1=mu[:],
                                        op=OP.subtract)
                nc.vector.tensor_tensor(out=xn_sb[:, ct], in0=xn_sb[:, ct], in1=rs[:],
                                        op=OP.mult)

        # ---------- phase 2: Q, K, V projections ----------
        with tc.tile_pool(name="wqkv_pool", bufs=1) as wp_pool:
            wqT_sb = wp_pool.tile([P, CT, CH], F32)
            wkT_sb = wp_pool.tile([P, CT, CH], F32)
            wvT_sb = wp_pool.tile([P, CT, CH], F32)
            nc.sync.dma_start(wqT_sb[:], wqT_d.rearrange("(ck p) d -> p ck d", p=P))
            nc.sync.dma_start(wkT_sb[:], wkT_d.rearrange("(ck p) d -> p ck d", p=P))
            nc.sync.dma_start(wvT_sb[:], wvT_d.rearrange("(ck p) d -> p ck d", p=P))

            for dt in range(CT):
                dsl = slice(dt * P, dt * P + P)
                for ic in range(IC):
                    sl = slice(ic * 512, ic * 512 + 512)
                    q_ps = psB.tile([P, 512], F32, tag="small")
                    for ck in range(CT):
                        nc.tensor.matmul(q_ps[:], lhsT=wqT_sb[:, ck, dsl],
                                         rhs=xn_sb[:, ck, sl],
                                         start=(ck == 0), stop=(ck == CT - 1))
                    nc.vector.tensor_scalar_add(out=qT_sb[:, dt, sl], in0=q_ps[:],
                                                scalar1=bqk_sb[:, 0, dt : dt + 1])
                    k_ps = psB.tile([P, 512], F32, tag="small")
                    for ck in range(CT):
                        nc.tensor.matmul(k_ps[:], lhsT=wkT_sb[:, ck, dsl],
                                         rhs=xn_sb[:, ck, sl],
                                         start=(ck == 0), stop=(ck == CT - 1))
                    nc.vector.tensor_scalar_add(out=kT_sb[:, dt, sl], in0=k_ps[:],
                                                scalar1=bqk_sb[:, 1, dt : dt + 1])

            for tt in range(TT):
                tsl = slice(tt * P, tt * P + P)
                v_ps = psB.tile([P, 512], F32, tag="small")
                for ck in range(CT):
                    nc.tensor.matmul(v_ps[:], lhsT=xn_sb[:, ck, tsl], rhs=wvT_sb[:, ck, :],
                                     start=(ck == 0), stop=False)
                # + bv_eff (K=1 ones-row matmul)
                nc.tensor.matmul(v_ps[:], lhsT=ones_row[:, :P], rhs=brow_sb[:, 0, :],
                                 start=False, stop=True)
                for h in range(HEADS):
                    nc.vector.tensor_copy(
                        out=v_sb[:, tt, h * (HD + 1) : h * (HD + 1) + HD],
                        in_=v_ps[:, h * HD : h * HD + HD])

        # ---------- phase 3: attention per head ----------
        for h in range(HEADS):
            dtl = h // 2
            drow = HD * (h % 2)
            strip = strip_pool.tile([P, STRIP_W], F32, tag="strip")
            nc.sync.dma_start(strip[:], strips_d[h])

            at_tiles = []
            for jt in range(TT):
                s_ps = psA.tile([P, NT], F32, tag="big")
                for ic in range(IC):
                    sl = slice(ic * 512, ic * 512 + 512)
                    nc.tensor.matmul(
                        s_ps[:, sl],
                        lhsT=kT_sb[drow : drow + HD, dtl, jt * P : jt * P + P],
                        rhs=qT_sb[drow : drow + HD, dtl, sl],
                        start=True, stop=True,
                    )
                off = (28 - 4 * jt) * 32
                nc.vector.tensor_tensor(out=s_ps[:], in0=s_ps[:],
                                        in1=strip[:, off : off + NT], op=OP.add)
                aT = at_pool.tile([P, NT], F32, name=f"aT_{h}_{jt}", tag="aT")
                nc.scalar.activation(out=aT[:], in_=s_ps[:], func=AF.Exp)
                at_tiles.append(aT)

            o_ps = ps_o.tile([HD + 1, NT], F32, tag="o")
            for jt in range(TT):
                for ic in range(IC):
                    sl = slice(ic * 512, ic * 512 + 512)
                    nc.tensor.matmul(
                        o_ps[:, sl],
                        lhsT=v_sb[:, jt, h * (HD + 1) : (h + 1) * (HD + 1)],
                        rhs=at_tiles[jt][:, sl],
                        start=(jt == 0), stop=(jt == TT - 1),
                    )
            # normalize rows 0..63 by row 64 (Z): replicate Z across partitions
            # via a K=1 matmul (DVE ops cannot partition-broadcast or shift)
            zrow = work.tile([P, NT], F32, tag="zrow")
            nc.vector.tensor_copy(out=zrow[HD : HD + 1, :], in_=o_ps[HD : HD + 1, :])
            zrep_ps = psA.tile([P, NT], F32, tag="big")
            for ic in range(IC):
                sl = slice(ic * 512, ic * 512 + 512)
                nc.tensor.matmul(zrep_ps[:HD, sl], lhsT=ones_mat[HD : HD + 1, :HD],
                                 rhs=zrow[HD : HD + 1, sl], start=True, stop=True)
            zrec = work.tile([P, NT], F32, tag="zrec")
            nc.vector.reciprocal(out=zrec[:HD, :], in_=zrep_ps[:HD, :])
            nc.vector.tensor_tensor(out=oT_sb[:, h], in0=o_ps[:HD, :], in1=zrec[:HD, :],
                                    op=OP.mult)

        # ---------- phase 4: output projection + residual ----------
        y_sb = singles.tile([P, CT, NT], F32)
        for ct in range(CT):
            csl = slice(ct * P, ct * P + P)
            for icc in range(IC):
                sl = slice(icc * 512, icc * 512 + 512)
                y_ps = psB.tile([P, 512], F32, tag="small")
                for h in range(HEADS):
                    nc.tensor.matmul(y_ps[:], lhsT=wpP_sb[:, h, csl],
                                     rhs=oT_sb[:, h, sl],
                                     start=(h == 0), stop=False)
                nc.tensor.matmul(y_ps[:], lhsT=brow_sb[:, 1, csl],
                                 rhs=ones_row[:, :512],
                                 start=False, stop=True)
                xres = work.tile([P, 512], F32, tag="xres")
                nc.sync.dma_start(xres[:], x_d[csl, sl])
                nc.vector.tensor_tensor(out=y_sb[:, ct, sl], in0=y_ps[:],
                                        in1=xres[:], op=OP.add)
            nc.sync.dma_start(y_d[csl, :], y_sb[:, ct])

    return nc


def _legalize_waits(nc, max_waits: int = 1):
    """Split multi-wait instructions into preceding same-engine NoOps.

    The TPB instruction encoding carries a single sync-wait slot and this
    walrus build refuses to legalize ("Too many sync wait commands"), so do
    it here: engines execute their queue in order, so a NoOp carrying one of
    the waits delays everything after it on that engine identically.
    """
    import orjson

    data = orjson.loads(mybir.module_to_json_bytes(nc.m))
    ctr = [0]

    def fix_block(block):
        out = []
        for inst in block.get("instructions", []):
            si = inst.get("sync_info") or {}
            waits = si.get("on_wait") or []
            if len(waits) > max_waits:
                for w in waits[max_waits:]:
                    ctr[0] += 1
                    nop = {
                        "name": f"I-WS{ctr[0]}",
                        "opcode": "NoOp",
                        "engine": inst["engine"],
                        "ins": [],
                        "outs": [],
                        "sync_info": {"on_wait": [w], "on_update": []},
                    }
                    if "debug" in inst:
                        nop["debug"] = inst["debug"]
                    out.append(nop)
                si = dict(si)
                si["on_wait"] = waits[:max_waits]
                inst["sync_info"] = si
            out.append(inst)
        block["instructions"] = out
        for b in block.get("blocks", []):
            fix_block(b)

    for fn in data["functions"]:
        for b in fn.get("blocks", []):
            fix_block(b)
    nc.m = mybir.module_from_json_bytes(orjson.dumps(data))
    return nc


_NC = None


def _host_prep(x, norm_w, norm_b, wq, bq, wk, bk, wv, bv, wp, bp, rel):
    scale = HD ** -0.5
    # fold LN affine + score scale into the projection weights (exact algebra)
    wq_eff = (wq * norm_w[None, :]) * scale
    bq_eff = (bq + wq @ norm_b) * scale
    wk_eff = wk * norm_w[None, :]
    bk_eff = bk + wk @ norm_b
    wv_eff = wv * norm_w[None, :]
    bv_eff = bv + wv @ norm_b

    wqT = np.ascontiguousarray(wq_eff.T).astype(np.float32)
    wkT = np.ascontiguousarray(wk_eff.T).astype(np.float32)
    wvT = np.ascontiguousarray(wv_eff.T).astype(np.float32)
    # wp permuted so each head's 64 input rows sit at partitions 0..63
    wpP = np.ascontiguousarray(
        wp.T.reshape(HEADS, HD, CH).transpose(1, 0, 2)
    ).astype(np.float32)

    bqk = np.stack([bq_eff, bk_eff]).astype(np.float32)
    brow = np.stack([bv_eff, bp]).astype(np.float32)
    strips = _build_strips(np.asarray(rel, np.float32))

    shared = {
        "wqT": wqT, "wkT": wkT, "wvT": wvT, "wpP": wpP,
        "bqk": bqk, "brow": brow, "strips": strips,
    }
    in_maps = []
    for b in range(B):
        m = dict(shared)
        m["x"] = np.ascontiguousarray(x[b].reshape(CH, NT)).astype(np.float32)
        in_maps.append(m)
    return in_maps


def kernel(**inputs):
    global _NC
    if _NC is None:
        _NC = _legalize_waits(_build_nc())
    in_maps = _host_prep(**{k: np.asarray(v) for k, v in inputs.items()})
    res = run_bass_kernel_spmd(_NC, in_maps, list(range(B)))
    out = np.stack([res.results[b]["y"].reshape(CH, H, W) for b in range(B)])
    return out.astype(np.float32)


if __name__ == "__main__":
    nc = _build_nc()
    print("built OK")



# revision 30
# speedup vs baseline: 2.6938x; 2.6938x over previous
"""Trainium2 Bass kernel for nn_MHAAttention (LayerNorm2d + MHA w/ rel-pos bias + residual).

Sharding: data-parallel over batch — 8 batch elements, one per NeuronCore.
No collectives needed.

Per-core pipeline (matmuls in bf16, accum fp32; residual path fp32):
  LN stats via ones-matmuls over host-sent bf16 x and x^2 (stats replicated
  across partitions by the M=128 ones stationary); rsqrt as exp(-0.5*ln(v+eps))
  so the Scalar engine stays in the one natural_log_exp table set.
  Q,K in (d part, t free); V in (t part, per-head [v|1] / [1|v] augment so the
  softmax denominator Z rides along the AV matmul and even/odd head outputs
  land on partitions 0-63 / 64-127 for pair-packed output projection.
  scores computed transposed per head (K=64 matmul), exp on ScalarE (scores
  are bounded, no max-subtraction), rel-pos bias applied multiplicatively:
  aT = exp(s) * exp(bias) with exp(bias) strips precomputed on host (bf16,
  2x-rate DVE multiply instead of a 1x PSUM add).
  Z inverted per head with reciprocal_approx_fast, replicated across
  partitions with a gpsimd partition_broadcast, multiplied on DVE.
  Output projection accumulates head-pairs (K=128), bias via K=1 ones-row
  matmul, residual add on DVE, DMA out per chunk.
"""

import sys

for _p in ("/opt/trn_rl_repo",):
    if _p not in sys.path:
        sys.path.insert(0, _p)

from contextlib import ExitStack

import ml_dtypes
import numpy as np

import concourse.bass as bass
import concourse.mybir as mybir
import concourse.tile as tile
from concourse import library_config
from concourse.bass_utils import run_bass_kernel_spmd

import os

USE_RECIP_FAST = os.environ.get("K_RECIP_FAST", "1") == "1"
# InstPartitionBroadcast fails walrus codegen ("ISA wrong length") on this
# build — default to the DMA 0-stride-source broadcast instead.
USE_PBCAST = os.environ.get("K_PBCAST", "0") == "1"

F32 = mybir.dt.float32
BF16 = mybir.dt.bfloat16
AF = mybir.ActivationFunctionType
OP = mybir.AluOpType
NPBF = ml_dtypes.bfloat16

B = 8
CH = 512
H = W = 32
NT = H * W          # 1024 tokens
HEADS = 8
HD = 64
HP = HEADS // 2     # head pairs
EPS = 1e-6
P = 128
CT = CH // P        # 4 channel tiles
TT = NT // P        # 8 token tiles
IC = NT // 512      # 2 free-dim chunks of 512
STRIP_W = 60 * 32   # 1920
VW = 128            # per-head v-aug width (padded so AV writes PSUM base 0)


def _build_strips(rel: np.ndarray) -> np.ndarray:
    """(3969, 8) rel table -> (8, 128, 1920) bias strips.

    strip[h, 32*jh_l + jw, 32*g + iw] = T_h[g - jh_l + 3, iw - jw + 31]
    where T_h = rel[:, h].reshape(63, 63).
    bias.T block for key-tile jt is then strip[:, (28-4*jt)*32 : +1024].
    """
    T = rel.reshape(63, 63, HEADS)  # [a, b, h]
    jh_l = np.arange(4)[:, None, None, None]
    jw = np.arange(32)[None, :, None, None]
    g = np.arange(60)[None, None, :, None]
    iw = np.arange(32)[None, None, None, :]
    a = g - jh_l + 3          # in [0,62]
    b = iw - jw + 31          # in [0,62]
    a_b, b_b = np.broadcast_arrays(a, b)
    out = T[a_b, b_b, :]      # (4, 32, 60, 32, 8)
    out = np.ascontiguousarray(np.moveaxis(out, -1, 0)).reshape(HEADS, 128, STRIP_W)
    return out.astype(np.float32)


def _build_nc() -> bass.Bass:
    nc = bass.Bass()

    x_d = nc.declare_dram_parameter("x", [CH, NT], F32, isOutput=False)
    xbf_d = nc.declare_dram_parameter("xbf", [CH, NT], BF16, isOutput=False)
    x2_d = nc.declare_dram_parameter("x2", [CH, NT], BF16, isOutput=False)
    wqT_d = nc.declare_dram_parameter("wqT", [CH, CH], BF16, isOutput=False)
    wkT_d = nc.declare_dram_parameter("wkT", [CH, CH], BF16, isOutput=False)
    wvT_d = nc.declare_dram_parameter("wvT", [CH, CH], BF16, isOutput=False)
    wpP_d = nc.declare_dram_parameter("wpP", [P, HP, CH], BF16, isOutput=False)
    bqk_d = nc.declare_dram_parameter("bqk", [2, CH], F32, isOutput=False)
    brow_d = nc.declare_dram_parameter("brow", [2, CH], BF16, isOutput=False)
    estrips_d = nc.declare_dram_parameter("estrips", [HEADS, P, STRIP_W], BF16,
                                          isOutput=False)
    y_d = nc.declare_dram_parameter("y", [CH, NT], F32, isOutput=True)

    with tile.TileContext(nc) as tc, ExitStack() as ctx:
        singles = ctx.enter_context(tc.tile_pool(name="singles", bufs=1))
        work = ctx.enter_context(tc.tile_pool(name="work", bufs=2))
        strip_pool = ctx.enter_context(tc.tile_pool(name="strip_pool", bufs=2))
        at_pool = ctx.enter_context(tc.tile_pool(name="at_pool", bufs=10))
        # PSUM (8 banks): psA big (128,1024)f32 x2bufs = 4 banks, lives the
        # whole kernel (LN stats -> scores -> proj partials via same tag).
        psA = ctx.enter_context(tc.tile_pool(name="psA", bufs=2, space="PSUM"))

        # ---------- persistent SBUF ----------
        x_sb = singles.tile([P, CT, NT], F32)        # residual source
        xbf_sb = singles.tile([P, CT, NT], BF16)
        xn_sb = singles.tile([P, CT, NT], BF16)      # LN output (matmul input)
        qT_sb = singles.tile([P, CT, NT], BF16)      # (d part, t free)
        kT_sb = singles.tile([P, CT, NT], BF16)
        v_sb = singles.tile([P, TT, HEADS * VW], BF16)
        oT_sb = singles.tile([P, HP, NT], BF16)      # head pairs packed
        wpP_sb = singles.tile([P, HP, CH], BF16)
        bqk_sb = singles.tile([P, 2, CT], F32)       # per-partition bias cols q,k
        brow_sb = singles.tile([1, 2, CH], BF16)     # bv_eff, bp rows
        ones_mat = singles.tile([P, P], BF16)
        ones_row = singles.tile([1, NT], BF16)
        mu_sb = singles.tile([P, NT], F32)
        rs_sb = singles.tile([P, NT], F32)

        if USE_PBCAST:
            # partition_broadcast + gpsimd tensor_tensor both live in 'proxy'
            nc.gpsimd.load_library(library_config.proxy)
        nc.vector.memset(ones_mat[:], 1.0)
        nc.vector.memset(ones_row[:], 1.0)
        nc.sync.dma_start(wpP_sb[:], wpP_d[:])
        nc.sync.dma_start(bqk_sb[:], bqk_d.rearrange("i (o p) -> p i o", p=P))
        nc.sync.dma_start(brow_sb[:], brow_d[None, :, :])

        # v_aug per head (128 wide): even = [v(64) | 1 | 0*63], odd =
        # [0*32 | 1 | 0*31 | v(64)] — AV output rows are 0-63/64-127 with the
        # Z row at 64/32 (engine ops need start partition in {0,32,64}), and
        # the matmul writes a base-0 full-128 PSUM block.
        v_view = v_sb[:].rearrange("p tt (h w) -> p tt h w", w=VW)
        nc.gpsimd.memset(v_sb[:], 0.0)
        for h in range(HEADS):
            oc = HD if h % 2 == 0 else HD // 2
            nc.vector.memset(v_view[:, :, h, oc : oc + 1], 1.0)

        nc.sync.dma_start(xbf_sb[:], xbf_d.rearrange("(ct p) t -> p ct t", p=P))
        nc.sync.dma_start(x_sb[:], x_d.rearrange("(ct p) t -> p ct t", p=P))

        # ---------- phase 1: LayerNorm stats + apply ----------
        with tc.tile_pool(name="ln_pool", bufs=1) as lnp, \
             tc.tile_pool(name="psB", bufs=2, space="PSUM") as psB:
            x2_sb = lnp.tile([P, CT, NT], BF16)
            nc.sync.dma_start(x2_sb[:], x2_d.rearrange("(ct p) t -> p ct t", p=P))

            sum_ps = psA.tile([P, NT], F32, tag="big")
            sq_ps = psA.tile([P, NT], F32, tag="big")
            for ct in range(CT):
                for ic in range(IC):
                    sl = slice(ic * 512, ic * 512 + 512)
                    nc.tensor.matmul(sum_ps[:, sl], lhsT=ones_mat[:],
                                     rhs=xbf_sb[:, ct, sl],
                                     start=(ct == 0), stop=(ct == CT - 1))
                    nc.tensor.matmul(sq_ps[:, sl], lhsT=ones_mat[:],
                                     rhs=x2_sb[:, ct, sl],
                                     start=(ct == 0), stop=(ct == CT - 1))

            ve = lnp.tile([P, NT], F32)
            m2 = lnp.tile([P, NT], F32)
            lnv = lnp.tile([P, NT], F32)
            nc.scalar.activation(out=mu_sb[:], in_=sum_ps[:], func=AF.Copy,
                                 scale=1.0 / CH)
            nc.vector.tensor_scalar(out=ve[:], in0=sq_ps[:], scalar1=1.0 / CH,
                                    scalar2=float(EPS), op0=OP.mult, op1=OP.add)
            nc.vector.tensor_tensor(out=m2[:], in0=mu_sb[:], in1=mu_sb[:],
                                    op=OP.mult)
            nc.vector.tensor_tensor(out=ve[:], in0=ve[:], in1=m2[:],
                                    op=OP.subtract)
            # rs = 1/sqrt(ve + eps) = exp(-0.5 * ln(ve + eps)); Ln and Exp share
            # one ACT table set so no table reload happens mid-kernel.
            nc.scalar.activation(out=lnv[:], in_=ve[:], func=AF.Ln)
            nc.scalar.activation(out=rs_sb[:], in_=lnv[:], func=AF.Exp, scale=-0.5)

            for ct in range(CT):
                xc = lnp.tile([P, NT], F32, name=f"xc_{ct}", tag="xc", bufs=2)
                nc.gpsimd.tensor_tensor(out=xc[:], in0=x_sb[:, ct], in1=mu_sb[:],
                                        op=OP.subtract)
                nc.vector.tensor_tensor(out=xn_sb[:, ct], in0=xc[:], in1=rs_sb[:],
                                        op=OP.mult)

            # ---------- phase 2: Q, K, V projections ----------
            with tc.tile_pool(name="wqkv_pool", bufs=1) as wp_pool:
                wqT_sb = wp_pool.tile([P, CT, CH], BF16)
                wkT_sb = wp_pool.tile([P, CT, CH], BF16)
                wvT_sb = wp_pool.tile([P, CT, CH], BF16)
                nc.sync.dma_start(wqT_sb[:], wqT_d.rearrange("(ck p) d -> p ck d", p=P))
                nc.sync.dma_start(wkT_sb[:], wkT_d.rearrange("(ck p) d -> p ck d", p=P))
                nc.sync.dma_start(wvT_sb[:], wvT_d.rearrange("(ck p) d -> p ck d", p=P))

                for dt in range(CT):
                    dsl = slice(dt * P, dt * P + P)
                    for ic in range(IC):
                        sl = slice(ic * 512, ic * 512 + 512)
                        q_ps = psB.tile([P, 512], F32, tag="small")
                        for ck in range(CT):
                            nc.tensor.matmul(q_ps[:], lhsT=wqT_sb[:, ck, dsl],
                                             rhs=xn_sb[:, ck, sl],
                                             start=(ck == 0), stop=(ck == CT - 1))
                        nc.vector.tensor_scalar_add(out=qT_sb[:, dt, sl],
                                                    in0=q_ps[:],
                                                    scalar1=bqk_sb[:, 0, dt : dt + 1])
                        k_ps = psB.tile([P, 512], F32, tag="small")
                        for ck in range(CT):
                            nc.tensor.matmul(k_ps[:], lhsT=wkT_sb[:, ck, dsl],
                                             rhs=xn_sb[:, ck, sl],
                                             start=(ck == 0), stop=(ck == CT - 1))
                        nc.vector.tensor_scalar_add(out=kT_sb[:, dt, sl],
                                                    in0=k_ps[:],
                                                    scalar1=bqk_sb[:, 1, dt : dt + 1])

                for tt in range(TT):
                    tsl = slice(tt * P, tt * P + P)
                    v_ps = psB.tile([P, 512], F32, tag="small")
                    for ck in range(CT):
                        nc.tensor.matmul(v_ps[:], lhsT=xn_sb[:, ck, tsl],
                                         rhs=wvT_sb[:, ck, :],
                                         start=(ck == 0), stop=False)
                    # + bv_eff (K=1 ones-row matmul)
                    nc.tensor.matmul(v_ps[:], lhsT=ones_row[:, :P],
                                     rhs=brow_sb[:, 0, :],
                                     start=False, stop=True)
                    # scatter per-head 64-wide blocks into the v-aug layout
                    # (even heads at cols 0-63 of their group, odd at 64-127)
                    vps_v = v_ps[:].rearrange("p (g hh d) -> p g hh d", hh=2, d=HD)
                    dst = v_view[:, tt]  # [p, h, VW]
                    dst_e = dst.rearrange("p (g hh) w -> p g hh w", hh=2)
                    nc.vector.tensor_copy(out=dst_e[:, :, 0, 0:HD],
                                          in_=vps_v[:, :, 0, :])
                    nc.vector.tensor_copy(out=dst_e[:, :, 1, HD:VW],
                                          in_=vps_v[:, :, 1, :])

        # ---------- phase 3: attention per head ----------
        with tc.tile_pool(name="ps_o", bufs=2, space="PSUM") as ps_o:
            for h in range(HEADS):
                dtl = h // 2
                drow = HD * (h % 2)
                even = h % 2 == 0
                strip = strip_pool.tile([P, STRIP_W], BF16, tag="strip")
                nc.sync.dma_start(strip[:], estrips_d[h])

                at_tiles = []
                for jt in range(TT):
                    s_ps = psA.tile([P, NT], F32, tag="big")
                    for ic in range(IC):
                        sl = slice(ic * 512, ic * 512 + 512)
                        nc.tensor.matmul(
                            s_ps[:, sl],
                            lhsT=kT_sb[drow : drow + HD, dtl, jt * P : jt * P + P],
                            rhs=qT_sb[drow : drow + HD, dtl, sl],
                            start=True, stop=True,
                        )
                    aT = at_pool.tile([P, NT], BF16, name=f"aT_{h}_{jt}", tag="aT")
                    nc.scalar.activation(out=aT[:], in_=s_ps[:], func=AF.Exp)
                    off = (28 - 4 * jt) * 32
                    nc.vector.tensor_tensor(out=aT[:], in0=aT[:],
                                            in1=strip[:, off : off + NT],
                                            op=OP.mult)
                    at_tiles.append(aT)

                o_ps = ps_o.tile([P, NT], F32, tag="o")
                # even head: rows 0..63 = oT, row 64 = Z
                # odd head: row 32 = Z, rows 64..127 = oT
                for jt in range(TT):
                    for ic in range(IC):
                        sl = slice(ic * 512, ic * 512 + 512)
                        nc.tensor.matmul(
                            o_ps[:, sl],
                            lhsT=v_sb[:, jt, h * VW : (h + 1) * VW],
                            rhs=at_tiles[jt][:, sl],
                            start=(jt == 0), stop=(jt == TT - 1),
                        )
                zrow = HD if even else HD // 2
                vlo = 0 if even else 64
                # 1/Z = exp(-ln(Z)) — both funcs live in the same ACT table
                # set as the attention exp, so no table reloads. Replicate
                # across the head's 64 partitions with a K=1 bf16 matmul.
                zln = work.tile([P, NT], F32, name=f"zln_{h}", tag="zln")
                nc.scalar.activation(out=zln[zrow : zrow + 1, :],
                                     in_=o_ps[zrow : zrow + 1, :], func=AF.Ln)
                zrec_bf = work.tile([P, NT], BF16, name=f"zrecb_{h}", tag="zrecb")
                nc.scalar.activation(out=zrec_bf[zrow : zrow + 1, :],
                                     in_=zln[zrow : zrow + 1, :], func=AF.Exp,
                                     scale=-1.0)
                zrep_ps = psA.tile([P, NT], F32, tag="big", name=f"zrep_{h}")
                for ic in range(IC):
                    sl = slice(ic * 512, ic * 512 + 512)
                    nc.tensor.matmul(
                        zrep_ps[vlo : vlo + HD, sl],
                        lhsT=ones_mat[zrow : zrow + 1, :HD],
                        rhs=zrec_bf[zrow : zrow + 1, sl],
                        start=True, stop=True)
                zb = work.tile([P, NT], BF16, name=f"zb_{h}", tag="zb")
                nc.scalar.activation(out=zb[vlo : vlo + HD, :],
                                     in_=zrep_ps[vlo : vlo + HD, :],
                                     func=AF.Identity)
                nc.vector.tensor_tensor(
                    out=oT_sb[vlo : vlo + HD, h // 2],
                    in0=o_ps[vlo : vlo + HD, :], in1=zb[vlo : vlo + HD, :],
                    op=OP.mult)

        # ---------- phase 4: output projection + residual ----------
        for ct in range(CT):
            csl = slice(ct * P, ct * P + P)
            for icc in range(IC):
                sl = slice(icc * 512, icc * 512 + 512)
                y_ps = psA.tile([P, 512], F32, tag="big", name=f"y_ps_{ct}_{icc}")
                for hp in range(HP):
                    nc.tensor.matmul(y_ps[:], lhsT=wpP_sb[:, hp, csl],
                                     rhs=oT_sb[:, hp, sl],
                                     start=(hp == 0), stop=False)
                nc.tensor.matmul(y_ps[:], lhsT=brow_sb[:, 1, csl],
                                 rhs=ones_row[:, :512],
                                 start=False, stop=True)
                y_sb = work.tile([P, 512], F32, name=f"y_sb_{ct}_{icc}", tag="ysb")
                nc.vector.tensor_tensor(out=y_sb[:], in0=y_ps[:],
                                        in1=x_sb[:, ct, sl], op=OP.add)
                nc.sync.dma_start(y_d[csl, sl], y_sb[:])

    return nc


def _legalize_waits(nc, max_waits: int = 1):
    """Split multi-wait instructions into preceding same-engine NoOps.

    The TPB instruction encoding carries a single sync-wait slot and this
    walrus build refuses to legalize ("Too many sync wait commands"), so do
    it here: engines execute their queue in order, so a NoOp carrying one of
    the waits delays everything after it on that engine identically.
    """
    import orjson

    data = orjson.loads(mybir.module_to_json_bytes(nc.m))
    ctr = [0]

    def fix_block(block):
        out = []
        for inst in block.get("instructions", []):
            si = inst.get("sync_info") or {}
            waits = si.get("on_wait") or []
            if len(waits) > max_waits:
                for w in waits[max_waits:]:
                    ctr[0] += 1
                    nop = {
                        "name": f"I-WS{ctr[0]}",
                        "opcode": "NoOp",
                        "engine": inst["engine"],
                        "ins": [],
                        "outs": [],
                        "sync_info": {"on_wait": [w], "on_update": []},
                    }
                    if "debug" in inst:
                        nop["debug"] = inst["debug"]
                    out.append(nop)
                si = dict(si)
                si["on_wait"] = waits[:max_waits]
                inst["sync_info"] = si
            out.append(inst)
        block["instructions"] = out
        for b in block.get("blocks", []):
            fix_block(b)

    for fn in data["functions"]:
        for b in fn.get("blocks", []):
            fix_block(b)
    nc.m = mybir.module_from_json_bytes(orjson.dumps(data))
    return nc


_NC = None


def _host_prep(x, norm_w, norm_b, wq, bq, wk, bk, wv, bv, wp, bp, rel):
    scale = HD ** -0.5
    # fold LN affine + score scale into the projection weights (exact algebra)
    wq_eff = (wq * norm_w[None, :]) * scale
    bq_eff = (bq + wq @ norm_b) * scale
    wk_eff = wk * norm_w[None, :]
    bk_eff = bk + wk @ norm_b
    wv_eff = wv * norm_w[None, :]
    bv_eff = bv + wv @ norm_b

    wqT = np.ascontiguousarray(wq_eff.T).astype(NPBF)
    wkT = np.ascontiguousarray(wk_eff.T).astype(NPBF)
    wvT = np.ascontiguousarray(wv_eff.T).astype(NPBF)
    # wp packed as head pairs: partitions 0-63 <- head 2hp, 64-127 <- head 2hp+1
    wpP = np.ascontiguousarray(
        wp.T.reshape(HP, 2 * HD, CH).transpose(1, 0, 2)
    ).astype(NPBF)

    bqk = np.stack([bq_eff, bk_eff]).astype(np.float32)
    brow = np.stack([bv_eff, bp]).astype(NPBF)
    estrips = np.exp(_build_strips(np.asarray(rel, np.float32))).astype(NPBF)

    shared = {
        "wqT": wqT, "wkT": wkT, "wvT": wvT, "wpP": wpP,
        "bqk": bqk, "brow": brow, "estrips": estrips,
    }
    in_maps = []
    for b in range(B):
        m = dict(shared)
        xb = np.ascontiguousarray(x[b].reshape(CH, NT)).astype(np.float32)
        m["x"] = xb
        m["xbf"] = xb.astype(NPBF)
        m["x2"] = (xb * xb).astype(NPBF)
        in_maps.append(m)
    return in_maps


def kernel(**inputs):
    global _NC
    if _NC is None:
        _NC = _legalize_waits(_build_nc())
    in_maps = _host_prep(**{k: np.asarray(v) for k, v in inputs.items()})
    res = run_bass_kernel_spmd(_NC, in_maps, list(range(B)))
    out = np.stack([res.results[b]["y"].reshape(CH, H, W) for b in range(B)])
    return out.astype(np.float32)


if __name__ == "__main__":
    nc = _build_nc()
    print("built OK")


# revision 32
# speedup vs baseline: 2.9517x; 1.0957x over previous
"""Trainium2 Bass kernel for nn_MHAAttention (LayerNorm2d + MHA w/ rel-pos bias + residual).

Sharding: data-parallel over batch — 8 batch elements, one per NeuronCore.
No collectives needed.

Per-core pipeline (matmuls in bf16, accum fp32; residual path fp32):
  LN stats via ones-matmuls over host-sent bf16 x and x^2 (stats replicated
  across partitions by the M=128 ones stationary); rsqrt as exp(-0.5*ln(v+eps))
  so the Scalar engine stays in the one natural_log_exp table set.
  Q,K in (d part, t free); V in (t part, per-head [v|1] / [1|v] augment so the
  softmax denominator Z rides along the AV matmul and even/odd head outputs
  land on partitions 0-63 / 64-127 for pair-packed output projection.
  scores computed transposed per head (K=64 matmul), exp on ScalarE (scores
  are bounded, no max-subtraction), rel-pos bias applied multiplicatively:
  aT = exp(s) * exp(bias) with exp(bias) strips precomputed on host (bf16,
  2x-rate DVE multiply instead of a 1x PSUM add).
  Z inverted per head with reciprocal_approx_fast, replicated across
  partitions with a gpsimd partition_broadcast, multiplied on DVE.
  Output projection accumulates head-pairs (K=128), bias via K=1 ones-row
  matmul, residual add on DVE, DMA out per chunk.
"""

import sys

for _p in ("/opt/trn_rl_repo",):
    if _p not in sys.path:
        sys.path.insert(0, _p)

from contextlib import ExitStack

import ml_dtypes
import numpy as np

import concourse.bass as bass
import concourse.mybir as mybir
import concourse.tile as tile
from concourse import library_config
from concourse.bass_utils import run_bass_kernel_spmd

import os

USE_RECIP_FAST = os.environ.get("K_RECIP_FAST", "1") == "1"
# InstPartitionBroadcast fails walrus codegen ("ISA wrong length") on this
# build — default to the DMA 0-stride-source broadcast instead.
USE_PBCAST = os.environ.get("K_PBCAST", "0") == "1"

F32 = mybir.dt.float32
BF16 = mybir.dt.bfloat16
AF = mybir.ActivationFunctionType
OP = mybir.AluOpType
NPBF = ml_dtypes.bfloat16

B = 8
CH = 512
H = W = 32
NT = H * W          # 1024 tokens
HEADS = 8
HD = 64
HP = HEADS // 2     # head pairs
EPS = 1e-6
P = 128
CT = CH // P        # 4 channel tiles
TT = NT // P        # 8 token tiles
IC = NT // 512      # 2 free-dim chunks of 512
STRIP_W = 60 * 32   # 1920
VW = 128            # per-head v-aug width (padded so AV writes PSUM base 0)


def _build_strips(rel: np.ndarray) -> np.ndarray:
    """(3969, 8) rel table -> (8, 128, 1920) bias strips.

    strip[h, 32*jh_l + jw, 32*g + iw] = T_h[g - jh_l + 3, iw - jw + 31]
    where T_h = rel[:, h].reshape(63, 63).
    bias.T block for key-tile jt is then strip[:, (28-4*jt)*32 : +1024].
    """
    T = rel.reshape(63, 63, HEADS)  # [a, b, h]
    jh_l = np.arange(4)[:, None, None, None]
    jw = np.arange(32)[None, :, None, None]
    g = np.arange(60)[None, None, :, None]
    iw = np.arange(32)[None, None, None, :]
    a = g - jh_l + 3          # in [0,62]
    b = iw - jw + 31          # in [0,62]
    a_b, b_b = np.broadcast_arrays(a, b)
    out = T[a_b, b_b, :]      # (4, 32, 60, 32, 8)
    out = np.ascontiguousarray(np.moveaxis(out, -1, 0)).reshape(HEADS, 128, STRIP_W)
    return out.astype(np.float32)


def _build_nc() -> bass.Bass:
    nc = bass.Bass()

    x_d = nc.declare_dram_parameter("x", [CH, NT], F32, isOutput=False)
    xbf_d = nc.declare_dram_parameter("xbf", [CH, NT], BF16, isOutput=False)
    x2_d = nc.declare_dram_parameter("x2", [CH, NT], BF16, isOutput=False)
    wqT_d = nc.declare_dram_parameter("wqT", [CH, CH], BF16, isOutput=False)
    wkT_d = nc.declare_dram_parameter("wkT", [CH, CH], BF16, isOutput=False)
    wvT_d = nc.declare_dram_parameter("wvT", [CH, CH], BF16, isOutput=False)
    wpP_d = nc.declare_dram_parameter("wpP", [P, HP, CH], BF16, isOutput=False)
    bqk_d = nc.declare_dram_parameter("bqk", [2, CH], F32, isOutput=False)
    brow_d = nc.declare_dram_parameter("brow", [2, CH], BF16, isOutput=False)
    estrips_d = nc.declare_dram_parameter("estrips", [HEADS, P, STRIP_W], BF16,
                                          isOutput=False)
    y_d = nc.declare_dram_parameter("y", [CH, NT], F32, isOutput=True)

    with tile.TileContext(nc) as tc, ExitStack() as ctx:
        singles = ctx.enter_context(tc.tile_pool(name="singles", bufs=1))
        work = ctx.enter_context(tc.tile_pool(name="work", bufs=2))
        strip_pool = ctx.enter_context(tc.tile_pool(name="strip_pool", bufs=2))
        at_pool = ctx.enter_context(tc.tile_pool(name="at_pool", bufs=10))
        # PSUM (8 banks): psA big (128,1024)f32 x2bufs = 4 banks, lives the
        # whole kernel (LN stats -> scores -> proj partials via same tag).
        psA = ctx.enter_context(tc.tile_pool(name="psA", bufs=2, space="PSUM"))

        # ---------- persistent SBUF ----------
        x_sb = singles.tile([P, CT, NT], F32)        # residual source
        xbf_sb = singles.tile([P, CT, NT], BF16)
        xn_sb = singles.tile([P, CT, NT], BF16)      # LN output (matmul input)
        qT_sb = singles.tile([P, CT, NT], BF16)      # (d part, t free)
        kT_sb = singles.tile([P, CT, NT], BF16)
        v_sb = singles.tile([P, TT, HEADS * VW], BF16)
        oT_sb = singles.tile([P, HP, NT], BF16)      # head pairs packed
        wpP_sb = singles.tile([P, HP, CH], BF16)
        bqk_sb = singles.tile([P, 2, CT], F32)       # per-partition bias cols q,k
        brow_sb = singles.tile([1, 2, CH], BF16)     # bv_eff, bp rows
        ones_mat = singles.tile([P, P], BF16)
        ones_row = singles.tile([1, NT], BF16)
        mu_sb = singles.tile([P, NT], F32)
        rs_sb = singles.tile([P, NT], F32)

        if USE_PBCAST:
            # partition_broadcast + gpsimd tensor_tensor both live in 'proxy'
            nc.gpsimd.load_library(library_config.proxy)
        nc.vector.memset(ones_mat[:], 1.0)
        nc.vector.memset(ones_row[:], 1.0)
        nc.sync.dma_start(wpP_sb[:], wpP_d[:])
        nc.sync.dma_start(bqk_sb[:], bqk_d.rearrange("i (o p) -> p i o", p=P))
        nc.sync.dma_start(brow_sb[:], brow_d[None, :, :])

        # v_aug per head (128 wide): even = [v(64) | 1 | 0*63], odd =
        # [0*32 | 1 | 0*31 | v(64)] — AV output rows are 0-63/64-127 with the
        # Z row at 64/32 (engine ops need start partition in {0,32,64}), and
        # the matmul writes a base-0 full-128 PSUM block.
        v_view = v_sb[:].rearrange("p tt (h w) -> p tt h w", w=VW)
        nc.gpsimd.memset(v_sb[:], 0.0)
        for h in range(HEADS):
            oc = HD if h % 2 == 0 else HD // 2
            nc.vector.memset(v_view[:, :, h, oc : oc + 1], 1.0)

        nc.sync.dma_start(xbf_sb[:], xbf_d.rearrange("(ct p) t -> p ct t", p=P))
        nc.sync.dma_start(x_sb[:], x_d.rearrange("(ct p) t -> p ct t", p=P))

        # ---------- phase 1: LayerNorm stats + apply ----------
        with tc.tile_pool(name="ln_pool", bufs=1) as lnp, \
             tc.tile_pool(name="psB", bufs=2, space="PSUM") as psB:
            x2_sb = lnp.tile([P, CT, NT], BF16)
            nc.sync.dma_start(x2_sb[:], x2_d.rearrange("(ct p) t -> p ct t", p=P))

            sum_ps = psA.tile([P, NT], F32, tag="big")
            sq_ps = psA.tile([P, NT], F32, tag="big")
            for ct in range(CT):
                for ic in range(IC):
                    sl = slice(ic * 512, ic * 512 + 512)
                    nc.tensor.matmul(sum_ps[:, sl], lhsT=ones_mat[:],
                                     rhs=xbf_sb[:, ct, sl],
                                     start=(ct == 0), stop=(ct == CT - 1))
                    nc.tensor.matmul(sq_ps[:, sl], lhsT=ones_mat[:],
                                     rhs=x2_sb[:, ct, sl],
                                     start=(ct == 0), stop=(ct == CT - 1))

            ve = lnp.tile([P, NT], F32)
            m2 = lnp.tile([P, NT], F32)
            lnv = lnp.tile([P, NT], F32)
            nc.scalar.activation(out=mu_sb[:], in_=sum_ps[:], func=AF.Copy,
                                 scale=1.0 / CH)
            nc.vector.tensor_scalar(out=ve[:], in0=sq_ps[:], scalar1=1.0 / CH,
                                    scalar2=float(EPS), op0=OP.mult, op1=OP.add)
            nc.vector.tensor_tensor(out=m2[:], in0=mu_sb[:], in1=mu_sb[:],
                                    op=OP.mult)
            nc.vector.tensor_tensor(out=ve[:], in0=ve[:], in1=m2[:],
                                    op=OP.subtract)
            # rs = 1/sqrt(ve + eps) = exp(-0.5 * ln(ve + eps)); Ln and Exp share
            # one ACT table set so no table reload happens mid-kernel.
            nc.scalar.activation(out=lnv[:], in_=ve[:], func=AF.Ln)
            nc.scalar.activation(out=rs_sb[:], in_=lnv[:], func=AF.Exp, scale=-0.5)

            for ct in range(CT):
                xc = lnp.tile([P, NT], F32, name=f"xc_{ct}", tag="xc", bufs=2)
                nc.gpsimd.tensor_tensor(out=xc[:], in0=x_sb[:, ct], in1=mu_sb[:],
                                        op=OP.subtract)
                nc.vector.tensor_tensor(out=xn_sb[:, ct], in0=xc[:], in1=rs_sb[:],
                                        op=OP.mult)

            # ---------- phase 2: Q, K, V projections ----------
            with tc.tile_pool(name="wqkv_pool", bufs=1) as wp_pool:
                wqT_sb = wp_pool.tile([P, CT, CH], BF16)
                wkT_sb = wp_pool.tile([P, CT, CH], BF16)
                wvT_sb = wp_pool.tile([P, CT, CH], BF16)
                nc.sync.dma_start(wqT_sb[:], wqT_d.rearrange("(ck p) d -> p ck d", p=P))
                nc.sync.dma_start(wkT_sb[:], wkT_d.rearrange("(ck p) d -> p ck d", p=P))
                nc.sync.dma_start(wvT_sb[:], wvT_d.rearrange("(ck p) d -> p ck d", p=P))

                for dt in range(CT):
                    dsl = slice(dt * P, dt * P + P)
                    for ic in range(IC):
                        sl = slice(ic * 512, ic * 512 + 512)
                        q_ps = psB.tile([P, 512], F32, tag="small")
                        for ck in range(CT):
                            nc.tensor.matmul(q_ps[:], lhsT=wqT_sb[:, ck, dsl],
                                             rhs=xn_sb[:, ck, sl],
                                             start=(ck == 0), stop=(ck == CT - 1))
                        nc.vector.tensor_scalar_add(out=qT_sb[:, dt, sl],
                                                    in0=q_ps[:],
                                                    scalar1=bqk_sb[:, 0, dt : dt + 1])
                        k_ps = psB.tile([P, 512], F32, tag="small")
                        for ck in range(CT):
                            nc.tensor.matmul(k_ps[:], lhsT=wkT_sb[:, ck, dsl],
                                             rhs=xn_sb[:, ck, sl],
                                             start=(ck == 0), stop=(ck == CT - 1))
                        nc.vector.tensor_scalar_add(out=kT_sb[:, dt, sl],
                                                    in0=k_ps[:],
                                                    scalar1=bqk_sb[:, 1, dt : dt + 1])

                for tt in range(TT):
                    tsl = slice(tt * P, tt * P + P)
                    v_ps = psB.tile([P, 512], F32, tag="small")
                    for ck in range(CT):
                        nc.tensor.matmul(v_ps[:], lhsT=xn_sb[:, ck, tsl],
                                         rhs=wvT_sb[:, ck, :],
                                         start=(ck == 0), stop=False)
                    # + bv_eff (K=1 ones-row matmul)
                    nc.tensor.matmul(v_ps[:], lhsT=ones_row[:, :P],
                                     rhs=brow_sb[:, 0, :],
                                     start=False, stop=True)
                    # scatter per-head 64-wide blocks into the v-aug layout
                    # (even heads at cols 0-63 of their group, odd at 64-127)
                    vps_v = v_ps[:].rearrange("p (g hh d) -> p g hh d", hh=2, d=HD)
                    dst = v_view[:, tt]  # [p, h, VW]
                    dst_e = dst.rearrange("p (g hh) w -> p g hh w", hh=2)
                    nc.vector.tensor_copy(out=dst_e[:, :, 0, 0:HD],
                                          in_=vps_v[:, :, 0, :])
                    nc.vector.tensor_copy(out=dst_e[:, :, 1, HD:VW],
                                          in_=vps_v[:, :, 1, :])

        # ---------- phase 3: attention, software-pipelined per head ----------
        # Per head: scores (PE) -> exp (ACT) -> bias-mult (DVE) -> AV (PE,
        # interleaved 2 jt behind scores so the PE queue never waits on ACT).
        # The Z-normalize chain (1/Z = exp(-ln Z) on ACT, partition broadcast
        # via a DRAM round-trip on the DMA engines, multiply on DVE) has no
        # PE instructions and is emitted one head late so no engine queue
        # ever stalls on it.
        zdram = nc.dram_tensor("zscratch", [HEADS, NT], BF16, kind="Internal")
        o_tiles = {}

        def z_start(h):
            even = h % 2 == 0
            zrow = HD if even else HD // 2
            vlo = 0 if even else 64
            o_ps = o_tiles[h]
            zln = work.tile([P, NT], F32, name=f"zln_{h}", tag="zln")
            nc.scalar.activation(out=zln[zrow : zrow + 1, :],
                                 in_=o_ps[zrow : zrow + 1, :], func=AF.Ln)
            zrec_bf = work.tile([P, NT], BF16, name=f"zrecb_{h}", tag="zrecb")
            nc.scalar.activation(out=zrec_bf[zrow : zrow + 1, :],
                                 in_=zln[zrow : zrow + 1, :], func=AF.Exp,
                                 scale=-1.0)
            nc.sync.dma_start(zdram[h : h + 1, :], zrec_bf[zrow : zrow + 1, :])
            zb = work.tile([P, NT], BF16, name=f"zb_{h}", tag="zb")
            nc.sync.dma_start(zb[vlo : vlo + HD, :],
                              zdram[h, :].partition_broadcast(HD))
            return zb

        def z_finish(h, zb):
            even = h % 2 == 0
            vlo = 0 if even else 64
            nc.vector.tensor_tensor(
                out=oT_sb[vlo : vlo + HD, h // 2],
                in0=o_tiles[h][vlo : vlo + HD, :], in1=zb[vlo : vlo + HD, :],
                op=OP.mult)

        with tc.tile_pool(name="ps_o", bufs=2, space="PSUM") as ps_o:
            strips = [None] * HEADS
            strips[0] = strip_pool.tile([P, STRIP_W], BF16, tag="strip",
                                        name="strip_0")
            nc.sync.dma_start(strips[0][:], estrips_d[0])
            zpend = None
            for h in range(HEADS):
                dtl = h // 2
                drow = HD * (h % 2)
                strip = strips[h]

                o_ps = ps_o.tile([P, NT], F32, tag="o", name=f"o_ps_{h}")
                o_tiles[h] = o_ps
                at_tiles = []

                def emit_av(jt):
                    for ic in range(IC):
                        sl = slice(ic * 512, ic * 512 + 512)
                        nc.tensor.matmul(
                            o_ps[:, sl],
                            lhsT=v_sb[:, jt, h * VW : (h + 1) * VW],
                            rhs=at_tiles[jt][:, sl],
                            start=(jt == 0), stop=(jt == TT - 1),
                        )

                for jt in range(TT):
                    s_ps = psA.tile([P, NT], F32, tag="big")
                    for ic in range(IC):
                        sl = slice(ic * 512, ic * 512 + 512)
                        nc.tensor.matmul(
                            s_ps[:, sl],
                            lhsT=kT_sb[drow : drow + HD, dtl, jt * P : jt * P + P],
                            rhs=qT_sb[drow : drow + HD, dtl, sl],
                            start=True, stop=True,
                        )
                    aT0 = at_pool.tile([P, NT], BF16, name=f"aT0_{h}_{jt}",
                                       tag="aT0", bufs=4)
                    nc.scalar.activation(out=aT0[:], in_=s_ps[:], func=AF.Exp)
                    off = (28 - 4 * jt) * 32
                    aT = at_pool.tile([P, NT], BF16, name=f"aT_{h}_{jt}", tag="aT")
                    nc.vector.tensor_tensor(out=aT[:], in0=aT0[:],
                                            in1=strip[:, off : off + NT],
                                            op=OP.mult)
                    at_tiles.append(aT)
                    if jt >= 2:
                        emit_av(jt - 2)
                    if jt == 0 and h + 1 < HEADS:
                        # prefetch next head's exp(bias) strip
                        strips[h + 1] = strip_pool.tile(
                            [P, STRIP_W], BF16, tag="strip", name=f"strip_{h+1}")
                        nc.sync.dma_start(strips[h + 1][:], estrips_d[h + 1])
                    if jt == 1 and zpend is not None:
                        z_finish(*zpend)
                        zpend = None
                emit_av(TT - 2)
                emit_av(TT - 1)
                zpend = (h, z_start(h))
            z_finish(*zpend)

        # ---------- phase 4: output projection + residual ----------
        for ct in range(CT):
            csl = slice(ct * P, ct * P + P)
            for icc in range(IC):
                sl = slice(icc * 512, icc * 512 + 512)
                y_ps = psA.tile([P, 512], F32, tag="big", name=f"y_ps_{ct}_{icc}")
                for hp in range(HP):
                    nc.tensor.matmul(y_ps[:], lhsT=wpP_sb[:, hp, csl],
                                     rhs=oT_sb[:, hp, sl],
                                     start=(hp == 0), stop=False)
                nc.tensor.matmul(y_ps[:], lhsT=brow_sb[:, 1, csl],
                                 rhs=ones_row[:, :512],
                                 start=False, stop=True)
                y_sb = work.tile([P, 512], F32, name=f"y_sb_{ct}_{icc}", tag="ysb")
                nc.vector.tensor_tensor(out=y_sb[:], in0=y_ps[:],
                                        in1=x_sb[:, ct, sl], op=OP.add)
                nc.sync.dma_start(y_d[csl, sl], y_sb[:])

    return nc


def _legalize_waits(nc, max_waits: int = 1):
    """Split multi-wait instructions into preceding same-engine NoOps.

    The TPB instruction encoding carries a single sync-wait slot and this
    walrus build refuses to legalize ("Too many sync wait commands"), so do
    it here: engines execute their queue in order, so a NoOp carrying one of
    the waits delays everything after it on that engine identically.
    """
    import orjson

    data = orjson.loads(mybir.module_to_json_bytes(nc.m))
    ctr = [0]

    def fix_block(block):
        out = []
        for inst in block.get("instructions", []):
            si = inst.get("sync_info") or {}
            waits = si.get("on_wait") or []
            if len(waits) > max_waits:
                for w in waits[max_waits:]:
                    ctr[0] += 1
                    nop = {
                        "name": f"I-WS{ctr[0]}",
                        "opcode": "NoOp",
                        "engine": inst["engine"],
                        "ins": [],
                        "outs": [],
                        "sync_info": {"on_wait": [w], "on_update": []},
                    }
                    if "debug" in inst:
                        nop["debug"] = inst["debug"]
                    out.append(nop)
                si = dict(si)
                si["on_wait"] = waits[:max_waits]
                inst["sync_info"] = si
            out.append(inst)
        block["instructions"] = out
        for b in block.get("blocks", []):
            fix_block(b)

    for fn in data["functions"]:
        for b in fn.get("blocks", []):
            fix_block(b)
    nc.m = mybir.module_from_json_bytes(orjson.dumps(data))
    return nc


_NC = None


def _host_prep(x, norm_w, norm_b, wq, bq, wk, bk, wv, bv, wp, bp, rel):
    scale = HD ** -0.5
    # fold LN affine + score scale into the projection weights (exact algebra)
    wq_eff = (wq * norm_w[None, :]) * scale
    bq_eff = (bq + wq @ norm_b) * scale
    wk_eff = wk * norm_w[None, :]
    bk_eff = bk + wk @ norm_b
    wv_eff = wv * norm_w[None, :]
    bv_eff = bv + wv @ norm_b

    wqT = np.ascontiguousarray(wq_eff.T).astype(NPBF)
    wkT = np.ascontiguousarray(wk_eff.T).astype(NPBF)
    wvT = np.ascontiguousarray(wv_eff.T).astype(NPBF)
    # wp packed as head pairs: partitions 0-63 <- head 2hp, 64-127 <- head 2hp+1
    wpP = np.ascontiguousarray(
        wp.T.reshape(HP, 2 * HD, CH).transpose(1, 0, 2)
    ).astype(NPBF)

    bqk = np.stack([bq_eff, bk_eff]).astype(np.float32)
    brow = np.stack([bv_eff, bp]).astype(NPBF)
    estrips = np.exp(_build_strips(np.asarray(rel, np.float32))).astype(NPBF)

    shared = {
        "wqT": wqT, "wkT": wkT, "wvT": wvT, "wpP": wpP,
        "bqk": bqk, "brow": brow, "estrips": estrips,
    }
    in_maps = []
    for b in range(B):
        m = dict(shared)
        xb = np.ascontiguousarray(x[b].reshape(CH, NT)).astype(np.float32)
        m["x"] = xb
        m["xbf"] = xb.astype(NPBF)
        m["x2"] = (xb * xb).astype(NPBF)
        in_maps.append(m)
    return in_maps


def kernel(**inputs):
    global _NC
    if _NC is None:
        _NC = _legalize_waits(_build_nc())
    in_maps = _host_prep(**{k: np.asarray(v) for k, v in inputs.items()})
    res = run_bass_kernel_spmd(_NC, in_maps, list(range(B)))
    out = np.stack([res.results[b]["y"].reshape(CH, H, W) for b in range(B)])
    return out.astype(np.float32)


if __name__ == "__main__":
    nc = _build_nc()
    print("built OK")
